# revision 54
# baseline (speedup 1.0000x reference)
"""kNN edge-feature kernel (PoseNet-style GNN message passing) for Trainium2.

Problem: given cloud [8, 3, 4096] f32, per batch element compute the K=16
nearest neighbors of every point (squared euclidean distance, self included)
and emit edge features [8, 6, 4096, 16]:
  out[b, 0:3, n, k] = cloud[b, :, n]                      (central, broadcast)
  out[b, 3:6, n, k] = cloud[b, :, idx[n,k]] - cloud[b, :, n]

Sharding: data-parallel over batch; core b handles batch element b.

Per-core pipeline, per 128-row tile (32 tiles):
  - negdist[n, m] = 2 x_n.x_m - |x_n|^2 - |x_m|^2 on the PE as a K=24 bf16
    contraction (bf16x3 error-compensated split of the fp32 inputs; bf16
    matmuls run 1 cycle/row vs 4 for fp32, and K<=128 is free). ~1e-6 noise.
  - pack pass (DVE scalar_tensor_tensor, one op per PSUM half):
    w = (negdist & ~511) | iota9 — the low 9 mantissa bits of each f32 are
    replaced with the chunk-local column index.  Ordering by w equals
    ordering by negdist except for values within ~2^-15 relative, every
    packed value is distinct, and the column comes back for free — no
    max_index pass over the distance matrix is ever needed.
  - top-8 per 512-column chunk on the DVE (max8) -> 64 candidates; then
    top-16 of 64 (max8 / match_replace / max8) + candidate slots via two
    small max_index ops; column = ((slot & ~7) << 6) | (w & 511).
  - neighbor xyz via gpsimd ap_gather, one call per xyz channel per group
    of GRP tiles from per-channel [128, 4096] broadcast tables (the same
    column list serves all three channels); the per-16-partition wrapped
    index semantics are undone by a constant mask multiply + pairwise
    add-tree, normally on gpsimd, but on the DVE for the last group where
    the DVE is otherwise drained (kills the end-of-kernel tail).
  - edge assembly with 6 ACT bias-broadcast ops per tile; strided store.

Engine budget per core (TimelineSim): DVE ~336us (bottleneck: pack 4.5us +
max8 4.75us + small top-k ops per tile), Pool ~241us, PE ~149us, ACT ~39us.
"""

import numpy as np

import concourse.bacc as bacc
import concourse.bass as bass
import concourse.mybir as mybir
from concourse.tile import TileContext

B, C, N, K = 8, 3, 4096, 16
P = 128            # rows per tile (SBUF partitions)
NT = N // P        # 32 row tiles
CH = 512           # chunk width for the per-chunk top-8
NCH = N // CH      # 8 chunks
KR = 24            # bf16x3 contraction depth
GRP = 4            # tiles batched per ap_gather call
NEG = -3.0e38      # match_replace sentinel
NE = C * N         # ap_gather table width (12288)
NIT = 48 * K       # gather list width per tile-batch: 48 idx/tile * 16 wrap

F32 = mybir.dt.float32
BF16 = mybir.dt.bfloat16
U32 = mybir.dt.uint32
U16 = mybir.dt.uint16
S16 = mybir.dt.int16

def build_program():
    nc = bacc.Bacc(trn_type="TRN2")
    lhs_d = nc.dram_tensor("lhs_aug", [KR, N], BF16, kind="ExternalInput")
    rhs_d = nc.dram_tensor("rhs_aug", [KR, N], BF16, kind="ExternalInput")
    bcast_d = nc.dram_tensor("bcast", [P, NE], F32, kind="ExternalInput")
    gmask_d = nc.dram_tensor("gmask", [P, GRP * K * K], F32, kind="ExternalInput")
    ctr_d = nc.dram_tensor("ctr_all", [P, NT * 2 * C], F32, kind="ExternalInput")
    out_d = nc.dram_tensor("out", [2 * C, N, K], F32, kind="ExternalOutput")

    with TileContext(nc) as tc:
        with (
            tc.tile_pool(name="persist", bufs=1) as persist,
            tc.tile_pool(name="wpool", bufs=3) as wpool,
            tc.tile_pool(name="nd", bufs=2) as ndpool,
            tc.tile_pool(name="mm", bufs=1, space="PSUM") as mmpool,
            tc.tile_pool(name="small", bufs=4) as small,
            tc.tile_pool(name="grp", bufs=2) as grppool,
            tc.tile_pool(name="gth", bufs=1) as gthpool,
        ):
            lhs_sb = persist.tile([KR, N], BF16)
            rhs_sb = persist.tile([KR, N], BF16)
            nc.sync.dma_start(lhs_sb[:], lhs_d[:])
            nc.sync.dma_start(rhs_sb[:], rhs_d[:])
            iota_sb = persist.tile([P, N], U32)
            _it = iota_sb[:]
            H2 = N // 2
            for ih in range(2):
                nc.gpsimd.iota(
                    bass.AP(
                        _it.tensor, _it.offset + ih * H2,
                        [_it.ap[0], [CH, H2 // CH], [1, CH]],
                    ),
                    pattern=[[0, H2 // CH], [1, CH]],
                    channel_multiplier=0,
                )
            cst_sb = persist.tile([P, 48], U32)
            nc.gpsimd.memset(cst_sb[:, 0:16], 511)
            nc.gpsimd.memset(cst_sb[:, 16:32], 0xFFF8)
            nc.gpsimd.memset(cst_sb[:, 32:48], 6)
            msk_sb = persist.tile([P, 1], U32)
            nc.gpsimd.memset(msk_sb[:], 0xFFFFFE00)
            msk2_sb = persist.tile([P, 1], U32)
            nc.gpsimd.memset(msk2_sb[:], 0xFFF8)
            msk3_sb = persist.tile([P, 1], U32)
            nc.gpsimd.memset(msk3_sb[:], 511)
            ctr_sb = persist.tile([P, NT * 2 * C], F32)
            nc.sync.dma_start(ctr_sb[:], ctr_d[:])
            bcast = persist.tile([P, NE], F32)
            nc.sync.dma_start(bcast[:], bcast_d[:])
            gmask = persist.tile([P, GRP * K * K], F32)
            nc.sync.dma_start(gmask[:], gmask_d[:])

            tbase = 0
            for gsz in [GRP] * (NT // GRP):
                idxb = grppool.tile([P, GRP * K], U16, tag="idxb")
                for tau in range(gsz):
                    t = tbase + tau
                    w = wpool.tile([P, N], U32, tag="w")
                    H = N // 2
                    for half in range(2):
                        # half PSUM (4 banks) per step; the DVE pack empties it
                        ps = mmpool.tile([P, H], F32, tag=f"ps{half}")
                        for j in range(H // CH):
                            nc.tensor.matmul(
                                ps[:, j * CH:(j + 1) * CH],
                                lhs_sb[:, t * P:(t + 1) * P],
                                rhs_sb[:, half * H + j * CH:half * H + (j + 1) * CH],
                                start=True,
                                stop=True,
                            )
                        # pack: w = (negdist & ~511) | iota9  (chunk-local column)
                        lo, hi = half * H, (half + 1) * H
                        nc.vector.scalar_tensor_tensor(
                            out=w[:, lo:hi],
                            in0=ps[:].bitcast(U32),
                            scalar=msk_sb[:, 0:1],
                            in1=iota_sb[:, lo:hi],
                            op0=mybir.AluOpType.bitwise_and,
                            op1=mybir.AluOpType.bitwise_or,
                        )
                        del ps

                    # stage 1: top-8 per 512-chunk (packed values)
                    cand = small.tile([P, NCH * 8], F32, tag="cand")
                    for c in range(NCH):
                        nc.vector.max(
                            out=cand[:, c * 8:(c + 1) * 8],
                            in_=w[:, c * CH:(c + 1) * CH].bitcast(F32),
                        )

                    # stage 2: top-16 of 64 + slots
                    w16 = small.tile([P, K], F32, tag="w16")
                    cand2 = small.tile([P, NCH * 8], F32, tag="cand2")
                    p16 = small.tile([P, K], U32, tag="p16")
                    nc.vector.max(out=w16[:, 0:8], in_=cand[:])
                    nc.vector.match_replace(
                        out=cand2[:], in_to_replace=w16[:, 0:8],
                        in_values=cand[:], imm_value=NEG,
                    )
                    nc.vector.max(out=w16[:, 8:16], in_=cand2[:])
                    nc.vector.max_index(
                        out=p16[:, 0:8], in_max=w16[:, 0:8], in_values=cand[:]
                    )
                    nc.vector.max_index(
                        out=p16[:, 8:16], in_max=w16[:, 8:16], in_values=cand2[:]
                    )

                    # columns: col = ((slot & ~7) << 6) | (w & 511)
                    tb = small.tile([P, K], U32, tag="tb")
                    nc.vector.scalar_tensor_tensor(
                        out=tb[:], in0=p16[:], scalar=msk2_sb[:, 0:1],
                        in1=cst_sb[:, 32:48],
                        op0=mybir.AluOpType.bitwise_and,
                        op1=mybir.AluOpType.logical_shift_left,
                    )
                    colw = small.tile([P, K], U32, tag="colw")
                    nc.vector.scalar_tensor_tensor(
                        out=colw[:], in0=w16[:].bitcast(U32),
                        scalar=msk3_sb[:, 0:1], in1=tb[:],
                        op0=mybir.AluOpType.bitwise_and,
                        op1=mybir.AluOpType.bitwise_or,
                    )
                    nc.vector.tensor_copy(idxb[:, tau * K:(tau + 1) * K], colw[:])

                # per-channel ap_gather from the broadcast table, then
                # mask + pairwise add-tree (all gpsimd) to undo the
                # 16-partition wrapped index semantics (baseline-proven).
                ni = gsz * K * K
                last = (tbase + gsz == NT)
                eng = nc.vector if last else nc.gpsimd
                nbrc = []
                for ch in range(C):
                    gth = gthpool.tile([P, GRP * K * K], F32,
                                       tag=f"gth{ch}", name=f"gth{ch}")
                    nc.gpsimd.ap_gather(
                        out_ap=gth[:, 0:ni],
                        in_ap=bcast[:, ch * N:(ch + 1) * N],
                        idxs_ap=idxb[:, 0:gsz * K].bitcast(S16),
                        channels=P,
                        num_elems=N,
                        d=1,
                        num_idxs=ni,
                    )
                    eng.tensor_tensor(
                        out=gth[:, 0:ni], in0=gth[:, 0:ni], in1=gmask[:, 0:ni],
                        op=mybir.AluOpType.mult,
                    )
                    seg = gsz * K
                    nbr1 = grppool.tile([P, GRP * K], F32,
                                        tag=f"nbr{ch}", name=f"nbr{ch}")
                    prev = gth
                    for lvl, wl in enumerate((K, K // 2, K // 4, K // 8)):
                        if wl > 2:
                            nxt = gthpool.tile(
                                [P, GRP * K * wl // 2], F32,
                                tag=f"tr{ch}_{lvl}", name=f"tr{ch}_{lvl}",
                            )
                        else:
                            nxt = nbr1
                        _p = prev[:]
                        a = bass.AP(_p.tensor, _p.offset,
                                    [_p.ap[0], [wl, seg], [2, wl // 2]])
                        b = bass.AP(_p.tensor, _p.offset + 1,
                                    [_p.ap[0], [wl, seg], [2, wl // 2]])
                        eng.tensor_tensor(
                            out=nxt[:, 0:seg * wl // 2].rearrange(
                                "p (s w) -> p s w", s=seg
                            ),
                            in0=a, in1=b, op=mybir.AluOpType.add,
                        )
                        prev = nxt
                    nbrc.append(nbr1)

                for tau in range(gsz):
                    t = tbase + tau
                    ot = small.tile([P, 2 * C, K], F32, tag="ot")
                    for c in range(C):
                        nb_c = nbrc[c][:, tau * K:(tau + 1) * K]
                        # neighbors - central: ACT bias-add
                        nc.scalar.activation(
                            ot[:, C + c, :], nb_c,
                            mybir.ActivationFunctionType.Identity,
                            bias=ctr_sb[:, t * 2 * C + C + c:t * 2 * C + C + c + 1],
                            scale=1.0,
                        )
                        # central broadcast
                        nc.scalar.activation(
                            ot[:, c, :], nb_c,
                            mybir.ActivationFunctionType.Identity,
                            bias=ctr_sb[:, t * 2 * C + c:t * 2 * C + c + 1],
                            scale=0.0,
                        )
                    nc.sync.dma_start(
                        out_d[:, t * P:(t + 1) * P, :].rearrange("c n k -> n c k"),
                        ot[:],
                    )
                tbase += gsz
    nc.compile()
    return nc


_nc_cache = None


def _get_nc():
    global _nc_cache
    if _nc_cache is None:
        _nc_cache = build_program()
    return _nc_cache


def _bf16_split3(x: np.ndarray):
    """x (f32) ~= h + m + l with each part exactly representable in bf16."""
    import ml_dtypes

    h = np.asarray(x, dtype=ml_dtypes.bfloat16)
    r = x - h.astype(np.float32)
    m = np.asarray(r, dtype=ml_dtypes.bfloat16)
    r2 = r - m.astype(np.float32)
    l = np.asarray(r2, dtype=ml_dtypes.bfloat16)
    return h, m, l


def make_in_maps(cloud: np.ndarray):
    import ml_dtypes

    cloud = np.ascontiguousarray(cloud, dtype=np.float32)
    assert cloud.shape == (B, C, N), cloud.shape

    q = np.arange(GRP * K * K) % K
    pm = np.arange(P)[:, None] % K
    gmask = (q[None, :] == pm).astype(np.float32)

    in_maps = []
    for b in range(B):
        x = cloud[b]                          # [3, N]
        sq = np.sum(x * x, axis=0, dtype=np.float32)   # [N]

        ah, am, al = _bf16_split3(2.0 * x)    # lhs side (2 x_n)
        bh, bm, bl = _bf16_split3(x)          # rhs side (x_m)
        sh, sm, sl = _bf16_split3(sq)

        bf = ml_dtypes.bfloat16
        ones = np.ones((C, N), dtype=bf)
        lhs = np.concatenate(
            [ah, ah, am, am, ah, al, -np.stack([sh, sm, sl]), ones]
        ).astype(bf)
        rhs = np.concatenate(
            [bh, bm, bh, bm, bl, bh, ones, np.stack([sh, sm, sl])]
        ).astype(bf)
        # sign fix: negdist = 2x.x' - sq_n - sq_m
        # block6 rows: lhs=-sq split, rhs=+1 ; block7: lhs=+1? -> need -sq_m
        lhs[21:24] = -ones
        assert lhs.shape == (KR, N) and rhs.shape == (KR, N)

        bcast = np.ascontiguousarray(
            np.broadcast_to(x.reshape(1, NE), (P, NE))
        )
        ctr_all = np.zeros((P, NT * 2 * C), np.float32)
        for t in range(NT):
            ctr_all[:, t * 2 * C:t * 2 * C + C] = x[:, t * P:(t + 1) * P].T
            ctr_all[:, t * 2 * C + C:(t + 1) * 2 * C] = -x[:, t * P:(t + 1) * P].T

        in_maps.append(
            {
                "lhs_aug": lhs,
                "rhs_aug": rhs,
                "bcast": bcast,
                "gmask": gmask,
                "ctr_all": ctr_all,
            }
        )
    return in_maps


_runner_cache = None


def _get_runner():
    """Cached jitted 8-core SPMD executor (mirrors bass2jax.run_bass_via_pjrt
    but reusable across calls so repeated runs don't re-trace)."""
    global _runner_cache
    if _runner_cache is not None:
        return _runner_cache

    import jax
    import numpy as _np
    from jax.sharding import Mesh, PartitionSpec
    from jax.experimental.shard_map import shard_map
    from concourse.bass2jax import (
        _bass_exec_p,
        install_neuronx_cc_hook,
        partition_id_tensor,
    )
    import concourse.mybir as _mybir

    nc = _get_nc()
    install_neuronx_cc_hook()
    partition_name = nc.partition_id_tensor.name if nc.partition_id_tensor else None

    in_names, out_names, out_avals, zero_outs = [], [], [], []
    for alloc in nc.m.functions[0].allocations:
        if not isinstance(alloc, _mybir.MemoryLocationSet):
            continue
        name = alloc.memorylocations[0].name
        if alloc.kind == "ExternalInput":
            if name != partition_name:
                in_names.append(name)
        elif alloc.kind == "ExternalOutput":
            shape = tuple(alloc.tensor_shape)
            dtype = _mybir.dt.np(alloc.dtype)
            out_names.append(name)
            out_avals.append(jax.core.ShapedArray(shape, dtype))
            zero_outs.append(_np.zeros(shape, dtype))
    n_params = len(in_names)
    n_outs = len(out_avals)
    all_in_names = list(in_names) + list(out_names)
    if partition_name is not None:
        all_in_names.append(partition_name)

    def _body(*args):
        operands = list(args)
        if partition_name is not None:
            operands.append(partition_id_tensor())
        outs = _bass_exec_p.bind(
            *operands,
            out_avals=tuple(out_avals),
            in_names=tuple(all_in_names),
            out_names=tuple(out_names),
            lowering_input_output_aliases=(),
            sim_require_finite=True,
            sim_require_nnan=True,
            nc=nc,
        )
        return tuple(outs)

    devices = jax.devices()[:B]
    mesh = Mesh(_np.asarray(devices), ("core",))
    in_specs = (PartitionSpec("core"),) * (n_params + n_outs)
    out_specs = (PartitionSpec("core"),) * n_outs
    sharded = jax.jit(
        shard_map(
            _body, mesh=mesh, in_specs=in_specs, out_specs=out_specs, check_rep=False
        ),
        keep_unused=True,
    )

    def runner(in_maps):
        per_core = [[np.asarray(m[name]) for name in in_names] for m in in_maps]
        concat_in = [
            np.concatenate([per_core[c][i] for c in range(B)], axis=0)
            for i in range(n_params)
        ]
        concat_zeros = [
            np.zeros((B * z.shape[0], *z.shape[1:]), z.dtype) for z in zero_outs
        ]
        out_arrs = sharded(*concat_in, *concat_zeros)
        return [
            {
                name: np.asarray(out_arrs[i]).reshape(B, *out_avals[i].shape)[c]
                for i, name in enumerate(out_names)
            }
            for c in range(B)
        ]

    _runner_cache = runner
    return runner


def run(cloud: np.ndarray):
    """Returns out [8, 6, 4096, 16] f32."""
    cloud = np.ascontiguousarray(cloud, dtype=np.float32)
    in_maps = make_in_maps(cloud)
    results = _get_runner()(in_maps)
    out = np.stack([r["out"] for r in results], axis=0)
    return out


def kernel(cloud: np.ndarray) -> np.ndarray:
    return run(cloud)


# revision 55
# speedup vs baseline: 1.0056x; 1.0056x over previous
"""kNN edge-feature kernel (PoseNet-style GNN message passing) for Trainium2.

Problem: given cloud [8, 3, 4096] f32, per batch element compute the K=16
nearest neighbors of every point (squared euclidean distance, self included)
and emit edge features [8, 6, 4096, 16]:
  out[b, 0:3, n, k] = cloud[b, :, n]                      (central, broadcast)
  out[b, 3:6, n, k] = cloud[b, :, idx[n,k]] - cloud[b, :, n]

Sharding: data-parallel over batch; core b handles batch element b.

Per-core pipeline, per 128-row tile (32 tiles):
  - negdist[n, m] = 2 x_n.x_m - |x_n|^2 - |x_m|^2 on the PE as a K=24 bf16
    contraction (bf16x3 error-compensated split of the fp32 inputs; bf16
    matmuls run 1 cycle/row vs 4 for fp32, and K<=128 is free). ~1e-6 noise.
  - pack pass (DVE scalar_tensor_tensor, one op per PSUM half):
    w = (negdist & ~511) | iota9 — the low 9 mantissa bits of each f32 are
    replaced with the chunk-local column index.  Ordering by w equals
    ordering by negdist except for values within ~2^-15 relative, every
    packed value is distinct, and the column comes back for free — no
    max_index pass over the distance matrix is ever needed.
  - top-8 per 512-column chunk on the DVE (max8) -> 64 candidates; then
    top-16 of 64 (max8 / match_replace / max8) + candidate slots via two
    small max_index ops; column = ((slot & ~7) << 6) | (w & 511).
  - neighbor xyz via gpsimd ap_gather, one call per xyz channel per group
    of GRP tiles from per-channel [128, 4096] broadcast tables (the same
    column list serves all three channels); the per-16-partition wrapped
    index semantics are undone by a constant mask multiply + pairwise
    add-tree, normally on gpsimd, but on the DVE for the last group where
    the DVE is otherwise drained (kills the end-of-kernel tail).
  - edge assembly with 6 ACT bias-broadcast ops per tile; strided store.

Engine budget per core (TimelineSim): DVE ~336us (bottleneck: pack 4.5us +
max8 4.75us + small top-k ops per tile), Pool ~241us, PE ~149us, ACT ~39us.
"""

import numpy as np

import concourse.bacc as bacc
import concourse.bass as bass
import concourse.mybir as mybir
from concourse.tile import TileContext

B, C, N, K = 8, 3, 4096, 16
P = 128            # rows per tile (SBUF partitions)
NT = N // P        # 32 row tiles
CH = 512           # chunk width for the per-chunk top-8
NCH = N // CH      # 8 chunks
KR = 24            # bf16x3 contraction depth
GRP = 6            # max tiles batched per ap_gather call
NEG = -3.0e38      # match_replace sentinel
NE = C * N         # ap_gather table width (12288)
NIT = 48 * K       # gather list width per tile-batch: 48 idx/tile * 16 wrap

F32 = mybir.dt.float32
BF16 = mybir.dt.bfloat16
U32 = mybir.dt.uint32
U16 = mybir.dt.uint16
S16 = mybir.dt.int16

def build_program():
    nc = bacc.Bacc(trn_type="TRN2")
    lhs_d = nc.dram_tensor("lhs_aug", [KR, N], BF16, kind="ExternalInput")
    rhs_d = nc.dram_tensor("rhs_aug", [KR, N], BF16, kind="ExternalInput")
    bcast_d = nc.dram_tensor("bcast", [P, NE], F32, kind="ExternalInput")
    gmask_d = nc.dram_tensor("gmask", [P, GRP * K * K], F32, kind="ExternalInput")
    ctr_d = nc.dram_tensor("ctr_all", [P, NT * 2 * C], F32, kind="ExternalInput")
    out_d = nc.dram_tensor("out", [2 * C, N, K], F32, kind="ExternalOutput")

    with TileContext(nc) as tc:
        with (
            tc.tile_pool(name="persist", bufs=1) as persist,
            tc.tile_pool(name="wpool", bufs=3) as wpool,
            tc.tile_pool(name="nd", bufs=2) as ndpool,
            tc.tile_pool(name="mm", bufs=1, space="PSUM") as mmpool,
            tc.tile_pool(name="small", bufs=6) as small,
            tc.tile_pool(name="grp", bufs=2) as grppool,
            tc.tile_pool(name="gth", bufs=1) as gthpool,
        ):
            lhs_sb = persist.tile([KR, N], BF16)
            rhs_sb = persist.tile([KR, N], BF16)
            nc.sync.dma_start(lhs_sb[:], lhs_d[:])
            nc.sync.dma_start(rhs_sb[:], rhs_d[:])
            iota_sb = persist.tile([P, N], U32)
            _it = iota_sb[:]
            H2 = N // 2
            for ih in range(2):
                nc.gpsimd.iota(
                    bass.AP(
                        _it.tensor, _it.offset + ih * H2,
                        [_it.ap[0], [CH, H2 // CH], [1, CH]],
                    ),
                    pattern=[[0, H2 // CH], [1, CH]],
                    channel_multiplier=0,
                )
            cst_sb = persist.tile([P, 48], U32)
            nc.gpsimd.memset(cst_sb[:, 0:16], 511)
            nc.gpsimd.memset(cst_sb[:, 16:32], 0xFFF8)
            nc.gpsimd.memset(cst_sb[:, 32:48], 6)
            msk_sb = persist.tile([P, 1], U32)
            nc.gpsimd.memset(msk_sb[:], 0xFFFFFE00)
            msk2_sb = persist.tile([P, 1], U32)
            nc.gpsimd.memset(msk2_sb[:], 0xFFF8)
            msk3_sb = persist.tile([P, 1], U32)
            nc.gpsimd.memset(msk3_sb[:], 511)
            ctr_sb = persist.tile([P, NT * 2 * C], F32)
            nc.sync.dma_start(ctr_sb[:], ctr_d[:])
            bcast = persist.tile([P, NE], F32)
            nc.sync.dma_start(bcast[:], bcast_d[:])
            gmask = persist.tile([P, GRP * K * K], F32)
            nc.sync.dma_start(gmask[:], gmask_d[:])

            tbase = 0
            for gsz in [6, 6, 6, 6, 4, 4]:
                idxb = grppool.tile([P, GRP * K], U16, tag="idxb")
                for tau in range(gsz):
                    t = tbase + tau
                    w = wpool.tile([P, N], U32, tag="w")
                    H = N // 2
                    for half in range(2):
                        # half PSUM (4 banks) per step; the DVE pack empties it
                        ps = mmpool.tile([P, H], F32, tag=f"ps{half}")
                        for j in range(H // CH):
                            nc.tensor.matmul(
                                ps[:, j * CH:(j + 1) * CH],
                                lhs_sb[:, t * P:(t + 1) * P],
                                rhs_sb[:, half * H + j * CH:half * H + (j + 1) * CH],
                                start=True,
                                stop=True,
                            )
                        # pack: w = (negdist & ~511) | iota9  (chunk-local column)
                        lo, hi = half * H, (half + 1) * H
                        nc.vector.scalar_tensor_tensor(
                            out=w[:, lo:hi],
                            in0=ps[:].bitcast(U32),
                            scalar=msk_sb[:, 0:1],
                            in1=iota_sb[:, lo:hi],
                            op0=mybir.AluOpType.bitwise_and,
                            op1=mybir.AluOpType.bitwise_or,
                        )
                        del ps

                    # stage 1: top-8 per 512-chunk (packed values)
                    cand = small.tile([P, NCH * 8], F32, tag="cand")
                    for c in range(NCH):
                        nc.vector.max(
                            out=cand[:, c * 8:(c + 1) * 8],
                            in_=w[:, c * CH:(c + 1) * CH].bitcast(F32),
                        )

                    # stage 2: top-16 of 64 + slots
                    w16 = small.tile([P, K], F32, tag="w16")
                    cand2 = small.tile([P, NCH * 8], F32, tag="cand2")
                    p16 = small.tile([P, K], U32, tag="p16")
                    nc.vector.max(out=w16[:, 0:8], in_=cand[:])
                    nc.vector.match_replace(
                        out=cand2[:], in_to_replace=w16[:, 0:8],
                        in_values=cand[:], imm_value=NEG,
                    )
                    nc.vector.max(out=w16[:, 8:16], in_=cand2[:])
                    nc.vector.max_index(
                        out=p16[:, 0:8], in_max=w16[:, 0:8], in_values=cand[:]
                    )
                    nc.vector.max_index(
                        out=p16[:, 8:16], in_max=w16[:, 8:16], in_values=cand2[:]
                    )

                    # columns: col = ((slot & ~7) << 6) | (w & 511)
                    tb = small.tile([P, K], U32, tag="tb")
                    nc.vector.scalar_tensor_tensor(
                        out=tb[:], in0=p16[:], scalar=msk2_sb[:, 0:1],
                        in1=cst_sb[:, 32:48],
                        op0=mybir.AluOpType.bitwise_and,
                        op1=mybir.AluOpType.logical_shift_left,
                    )
                    colw = small.tile([P, K], U32, tag="colw")
                    nc.vector.scalar_tensor_tensor(
                        out=colw[:], in0=w16[:].bitcast(U32),
                        scalar=msk3_sb[:, 0:1], in1=tb[:],
                        op0=mybir.AluOpType.bitwise_and,
                        op1=mybir.AluOpType.bitwise_or,
                    )
                    nc.gpsimd.tensor_copy(idxb[:, tau * K:(tau + 1) * K], colw[:])

                # per-channel ap_gather from the broadcast table, then
                # mask + pairwise add-tree (all gpsimd) to undo the
                # 16-partition wrapped index semantics (baseline-proven).
                ni = gsz * K * K
                last = (tbase + gsz == NT)
                eng = nc.vector if last else nc.gpsimd
                nbrc = []
                for ch in range(C):
                    gth = gthpool.tile([P, GRP * K * K], F32,
                                       tag=f"gth{ch}", name=f"gth{ch}")
                    nc.gpsimd.ap_gather(
                        out_ap=gth[:, 0:ni],
                        in_ap=bcast[:, ch * N:(ch + 1) * N],
                        idxs_ap=idxb[:, 0:gsz * K].bitcast(S16),
                        channels=P,
                        num_elems=N,
                        d=1,
                        num_idxs=ni,
                    )
                    eng.tensor_tensor(
                        out=gth[:, 0:ni], in0=gth[:, 0:ni], in1=gmask[:, 0:ni],
                        op=mybir.AluOpType.mult,
                    )
                    seg = gsz * K
                    nbr1 = grppool.tile([P, GRP * K], F32,
                                        tag=f"nbr{ch}", name=f"nbr{ch}")
                    prev = gth
                    for lvl, wl in enumerate((K, K // 2, K // 4, K // 8)):
                        if wl > 2:
                            nxt = gthpool.tile(
                                [P, GRP * K * wl // 2], F32,
                                tag=f"tr{ch}_{lvl}", name=f"tr{ch}_{lvl}",
                            )
                        else:
                            nxt = nbr1
                        _p = prev[:]
                        a = bass.AP(_p.tensor, _p.offset,
                                    [_p.ap[0], [wl, seg], [2, wl // 2]])
                        b = bass.AP(_p.tensor, _p.offset + 1,
                                    [_p.ap[0], [wl, seg], [2, wl // 2]])
                        eng.tensor_tensor(
                            out=nxt[:, 0:seg * wl // 2].rearrange(
                                "p (s w) -> p s w", s=seg
                            ),
                            in0=a, in1=b, op=mybir.AluOpType.add,
                        )
                        prev = nxt
                    nbrc.append(nbr1)

                for tau in range(gsz):
                    t = tbase + tau
                    ot = small.tile([P, 2 * C, K], F32, tag="ot")
                    for c in range(C):
                        nb_c = nbrc[c][:, tau * K:(tau + 1) * K]
                        # neighbors - central: ACT bias-add
                        nc.scalar.activation(
                            ot[:, C + c, :], nb_c,
                            mybir.ActivationFunctionType.Identity,
                            bias=ctr_sb[:, t * 2 * C + C + c:t * 2 * C + C + c + 1],
                            scale=1.0,
                        )
                        # central broadcast
                        nc.scalar.activation(
                            ot[:, c, :], nb_c,
                            mybir.ActivationFunctionType.Identity,
                            bias=ctr_sb[:, t * 2 * C + c:t * 2 * C + c + 1],
                            scale=0.0,
                        )
                    nc.sync.dma_start(
                        out_d[:, t * P:(t + 1) * P, :].rearrange("c n k -> n c k"),
                        ot[:],
                    )
                tbase += gsz
    nc.compile()
    return nc


_nc_cache = None


def _get_nc():
    global _nc_cache
    if _nc_cache is None:
        _nc_cache = build_program()
    return _nc_cache


def _bf16_split3(x: np.ndarray):
    """x (f32) ~= h + m + l with each part exactly representable in bf16."""
    import ml_dtypes

    h = np.asarray(x, dtype=ml_dtypes.bfloat16)
    r = x - h.astype(np.float32)
    m = np.asarray(r, dtype=ml_dtypes.bfloat16)
    r2 = r - m.astype(np.float32)
    l = np.asarray(r2, dtype=ml_dtypes.bfloat16)
    return h, m, l


def make_in_maps(cloud: np.ndarray):
    import ml_dtypes

    cloud = np.ascontiguousarray(cloud, dtype=np.float32)
    assert cloud.shape == (B, C, N), cloud.shape

    q = np.arange(GRP * K * K) % K
    pm = np.arange(P)[:, None] % K
    gmask = (q[None, :] == pm).astype(np.float32)

    in_maps = []
    for b in range(B):
        x = cloud[b]                          # [3, N]
        sq = np.sum(x * x, axis=0, dtype=np.float32)   # [N]

        ah, am, al = _bf16_split3(2.0 * x)    # lhs side (2 x_n)
        bh, bm, bl = _bf16_split3(x)          # rhs side (x_m)
        sh, sm, sl = _bf16_split3(sq)

        bf = ml_dtypes.bfloat16
        ones = np.ones((C, N), dtype=bf)
        lhs = np.concatenate(
            [ah, ah, am, am, ah, al, -np.stack([sh, sm, sl]), ones]
        ).astype(bf)
        rhs = np.concatenate(
            [bh, bm, bh, bm, bl, bh, ones, np.stack([sh, sm, sl])]
        ).astype(bf)
        # sign fix: negdist = 2x.x' - sq_n - sq_m
        # block6 rows: lhs=-sq split, rhs=+1 ; block7: lhs=+1? -> need -sq_m
        lhs[21:24] = -ones
        assert lhs.shape == (KR, N) and rhs.shape == (KR, N)

        bcast = np.ascontiguousarray(
            np.broadcast_to(x.reshape(1, NE), (P, NE))
        )
        ctr_all = np.zeros((P, NT * 2 * C), np.float32)
        for t in range(NT):
            ctr_all[:, t * 2 * C:t * 2 * C + C] = x[:, t * P:(t + 1) * P].T
            ctr_all[:, t * 2 * C + C:(t + 1) * 2 * C] = -x[:, t * P:(t + 1) * P].T

        in_maps.append(
            {
                "lhs_aug": lhs,
                "rhs_aug": rhs,
                "bcast": bcast,
                "gmask": gmask,
                "ctr_all": ctr_all,
            }
        )
    return in_maps


_runner_cache = None


def _get_runner():
    """Cached jitted 8-core SPMD executor (mirrors bass2jax.run_bass_via_pjrt
    but reusable across calls so repeated runs don't re-trace)."""
    global _runner_cache
    if _runner_cache is not None:
        return _runner_cache

    import jax
    import numpy as _np
    from jax.sharding import Mesh, PartitionSpec
    from jax.experimental.shard_map import shard_map
    from concourse.bass2jax import (
        _bass_exec_p,
        install_neuronx_cc_hook,
        partition_id_tensor,
    )
    import concourse.mybir as _mybir

    nc = _get_nc()
    install_neuronx_cc_hook()
    partition_name = nc.partition_id_tensor.name if nc.partition_id_tensor else None

    in_names, out_names, out_avals, zero_outs = [], [], [], []
    for alloc in nc.m.functions[0].allocations:
        if not isinstance(alloc, _mybir.MemoryLocationSet):
            continue
        name = alloc.memorylocations[0].name
        if alloc.kind == "ExternalInput":
            if name != partition_name:
                in_names.append(name)
        elif alloc.kind == "ExternalOutput":
            shape = tuple(alloc.tensor_shape)
            dtype = _mybir.dt.np(alloc.dtype)
            out_names.append(name)
            out_avals.append(jax.core.ShapedArray(shape, dtype))
            zero_outs.append(_np.zeros(shape, dtype))
    n_params = len(in_names)
    n_outs = len(out_avals)
    all_in_names = list(in_names) + list(out_names)
    if partition_name is not None:
        all_in_names.append(partition_name)

    def _body(*args):
        operands = list(args)
        if partition_name is not None:
            operands.append(partition_id_tensor())
        outs = _bass_exec_p.bind(
            *operands,
            out_avals=tuple(out_avals),
            in_names=tuple(all_in_names),
            out_names=tuple(out_names),
            lowering_input_output_aliases=(),
            sim_require_finite=True,
            sim_require_nnan=True,
            nc=nc,
        )
        return tuple(outs)

    devices = jax.devices()[:B]
    mesh = Mesh(_np.asarray(devices), ("core",))
    in_specs = (PartitionSpec("core"),) * (n_params + n_outs)
    out_specs = (PartitionSpec("core"),) * n_outs
    sharded = jax.jit(
        shard_map(
            _body, mesh=mesh, in_specs=in_specs, out_specs=out_specs, check_rep=False
        ),
        keep_unused=True,
    )

    def runner(in_maps):
        per_core = [[np.asarray(m[name]) for name in in_names] for m in in_maps]
        concat_in = [
            np.concatenate([per_core[c][i] for c in range(B)], axis=0)
            for i in range(n_params)
        ]
        concat_zeros = [
            np.zeros((B * z.shape[0], *z.shape[1:]), z.dtype) for z in zero_outs
        ]
        out_arrs = sharded(*concat_in, *concat_zeros)
        return [
            {
                name: np.asarray(out_arrs[i]).reshape(B, *out_avals[i].shape)[c]
                for i, name in enumerate(out_names)
            }
            for c in range(B)
        ]

    _runner_cache = runner
    return runner


def run(cloud: np.ndarray):
    """Returns out [8, 6, 4096, 16] f32."""
    cloud = np.ascontiguousarray(cloud, dtype=np.float32)
    in_maps = make_in_maps(cloud)
    results = _get_runner()(in_maps)
    out = np.stack([r["out"] for r in results], axis=0)
    return out


def kernel(cloud: np.ndarray) -> np.ndarray:
    return run(cloud)
